# revision 7
# baseline (speedup 1.0000x reference)
"""Trainium2 Bass kernel for nn_NewModel_66176856097442 (TransE-style loss).

Strategy (data-parallel over the batch of triples):
  - B = 262144 triples sharded as 32768/core across 8 NeuronCores,
    laid out [128 partitions x 256 columns] per core.
  - Entity table fused on host into fp16 rows of 152 elems (304B):
      [ vec(128) | bias | ||vec||^2 | R(18) | pad(4) ]
    where R[e,k] = vec[e] . relEmb[k].  With per-entity norms and R, no
    per-triple elementwise vector arithmetic is needed:
      ||a - b||^2           = n_a + n_b - 2 a.b
      ||a + r_rel - b||^2   = ||a-b||^2 + 2(R[a,rel] - R[b,rel]) + ||r_rel||^2
    so the only per-triple vector work is the three cross dots
    (lv.rv, nlv.rv, lv.nrv), each a fused mul+reduce (tensor_tensor_reduce)
    per 128-triple column.
  - Rel-side per-triple meta (masks, ||relEmb||^2, onehot(18)) is expanded on
    host (18-entry lookup) and uploaded as one sequential tile per core.
  - Gathers: one gpsimd indirect DMA per (array, column) writing into a
    chunk-sized buffer slice; hardware only supports one index per partition
    per indirect DMA, so 4*256 calls/core.  Compute is batched per chunk of
    CH columns on large strided views.
  - Final margin/mask algebra runs once on [128, 256] tiles; per-core
    partial sum returned as [128,1]; host sums / B.
"""

import sys

sys.path.insert(0, "/opt/trn_rl_repo")

import numpy as np

import concourse.bass as bass
from concourse import bacc
import concourse.tile as tile
from concourse import mybir
from concourse.bass import IndirectOffsetOnAxis
from concourse.bass_utils import run_bass_kernel_spmd

F32 = mybir.dt.float32
F16 = mybir.dt.float16
I32 = mybir.dt.int32
ALU = mybir.AluOpType
AX = mybir.AxisListType

NUM_ENTITY = 100000
NUM_RELATION = 18
D = 128
B = 262144
N_CORES = 8
NB = B // N_CORES          # triples per core (32768)
P = 128                    # partitions
NBK = NB // P              # triples per partition per core (256)
CH = 32                    # columns per compute chunk
MARGIN = 1.0

# fused entity row: [vec(128) | bias | norm | R(18) | pad] -> 152 fp16 = 304B
EW = 152
OFF_BIAS = 128
OFF_NORM = 129
OFF_R = 130
# host-expanded rel meta per triple: [mh | mr | ms | nre2 | onehot(18) | pad]
RW = 24
OFF_H = 4

HYPONYM = (4, 6)
HYPERNYM = (3, 5)
SYNONYM = (0, 1, 13, 17)


def build_bass(nb=NB):
    """Per-core Bass kernel; nb = triples handled by this core."""
    nbk = nb // P
    assert nbk % CH == 0
    nch = nbk // CH

    nc = bacc.Bacc("TRN2", target_bir_lowering=False, debug=True)

    vec_t = nc.declare_dram_parameter("vec", [NUM_ENTITY, EW], F16, isOutput=False)
    relm_t = nc.declare_dram_parameter("relmeta", [P, nbk * RW], F16, isOutput=False)
    li_t = nc.declare_dram_parameter("li", [P, nbk], I32, isOutput=False)
    ri_t = nc.declare_dram_parameter("ri", [P, nbk], I32, isOutput=False)
    nli_t = nc.declare_dram_parameter("nli", [P, nbk], I32, isOutput=False)
    nri_t = nc.declare_dram_parameter("nri", [P, nbk], I32, isOutput=False)
    out_t = nc.declare_dram_parameter("psum_out", [P, 1], F32, isOutput=True)

    with tile.TileContext(nc) as tc:
        with (
            tc.tile_pool(name="persist", bufs=1) as persist,
            tc.tile_pool(name="gather", bufs=2) as gpool,
            tc.tile_pool(name="scratch", bufs=2) as spool,
            tc.tile_pool(name="final", bufs=1) as fpool,
        ):
            # ---- load index arrays + rel meta to SBUF once ----
            li = persist.tile([P, nbk], I32, name="li")
            ri = persist.tile([P, nbk], I32, name="ri")
            nli = persist.tile([P, nbk], I32, name="nli")
            nri = persist.tile([P, nbk], I32, name="nri")
            relm = persist.tile([P, nbk * RW], F16, name="relm")
            nc.sync.dma_start(out=li[:], in_=li_t[:])
            nc.sync.dma_start(out=ri[:], in_=ri_t[:])
            nc.sync.dma_start(out=nli[:], in_=nli_t[:])
            nc.sync.dma_start(out=nri[:], in_=nri_t[:])
            nc.sync.dma_start(out=relm[:], in_=relm_t[:])
            relm3 = relm[:].rearrange("p (c w) -> p c w", c=nbk, w=RW)

            # per-triple accumulators (f32) and extracted scalars (f16)
            dots = [persist.tile([P, nbk], F32, name=f"d{k}") for k in range(3)]
            conts = [persist.tile([P, nbk], F32, name=f"c{k}") for k in range(4)]
            exn = [persist.tile([P, nbk], F16, name=f"n{k}") for k in range(4)]
            exb = [persist.tile([P, nbk], F16, name=f"b{k}") for k in range(4)]

            idxs = (li, ri, nli, nri)
            for c in range(nch):
                j0 = c * CH
                jsl = slice(j0, j0 + CH)
                gts = []
                for a, (ixt, anm) in enumerate(zip(idxs, ("lv", "rv", "nlv", "nrv"))):
                    gt = gpool.tile([P, CH * EW], F16, name=anm, tag=anm)
                    for j in range(CH):
                        nc.gpsimd.indirect_dma_start(
                            out=gt[:, j * EW : (j + 1) * EW],
                            out_offset=None,
                            in_=vec_t[:],
                            in_offset=IndirectOffsetOnAxis(
                                ap=ixt[:, j0 + j : j0 + j + 1], axis=0
                            ),
                        )
                    gts.append(gt[:].rearrange("p (c w) -> p c w", c=CH, w=EW))

                # ---- extract per-triple scalars into packed persist tiles ----
                for a in range(4):
                    nc.vector.tensor_copy(
                        exn[a][:, jsl],
                        gts[a][:, :, OFF_NORM : OFF_NORM + 1].squeeze(),
                    )
                    nc.vector.tensor_copy(
                        exb[a][:, jsl],
                        gts[a][:, :, OFF_BIAS : OFF_BIAS + 1].squeeze(),
                    )

                # ---- cross dots: batched mul + 3D-strided reduce per pair ----
                for k, (a, b) in enumerate(((0, 1), (2, 1), (0, 3))):
                    prod = spool.tile([P, CH * D], F16, name=f"dp{k}", tag=f"dp{k}")
                    dp3 = prod[:].rearrange("p (c w) -> p c w", c=CH, w=D)
                    nc.vector.tensor_tensor(
                        out=dp3,
                        in0=gts[a][:, :, 0:D],
                        in1=gts[b][:, :, 0:D],
                        op=ALU.mult,
                    )
                    nc.vector.tensor_reduce(
                        out=dots[k][:, jsl], in_=dp3, axis=AX.X, op=ALU.add
                    )

                # ---- onehot . R contraction per entity array ----
                hview = relm3[:, jsl, OFF_H : OFF_H + NUM_RELATION]
                for a in range(4):
                    prod = spool.tile(
                        [P, CH * NUM_RELATION], F16, name=f"pr{a}", tag=f"pr{a}"
                    )
                    p3 = prod[:].rearrange("p (c w) -> p c w", c=CH, w=NUM_RELATION)
                    nc.vector.tensor_tensor(
                        out=p3,
                        in0=hview,
                        in1=gts[a][:, :, OFF_R : OFF_R + NUM_RELATION],
                        op=ALU.mult,
                    )
                    nc.vector.tensor_reduce(
                        out=conts[a][:, jsl], in_=p3, axis=AX.X, op=ALU.add
                    )

            # ================= final phase on [P, nbk] tiles =================
            f = lambda nm: fpool.tile([P, nbk], F32, name=nm)

            nl_, nr_, nnl, nnr = (t[:] for t in exn)
            bl, br, bnl, bnr = (t[:] for t in exb)
            mh = relm3[:, :, 0:1].squeeze()
            mr = relm3[:, :, 1:2].squeeze()
            ms = relm3[:, :, 2:3].squeeze()
            q = relm3[:, :, 3:4].squeeze()
            cl, cr, cnl, cnr = (t[:] for t in conts)

            # s_k = n_a + n_b - 2 d_k   (clamped at 0)
            s_list, vd_list, tr_list = [], [], []
            for k, (na, nb_) in enumerate(((nl_, nr_), (nnl, nr_), (nl_, nnr))):
                sk = f(f"s{k}")
                nc.vector.tensor_tensor(out=sk, in0=na, in1=nb_, op=ALU.add)
                nc.vector.scalar_tensor_tensor(
                    sk, dots[k][:], -2.0, sk, op0=ALU.mult, op1=ALU.add
                )
                nc.vector.tensor_scalar_max(sk, sk, 0.0)
                s_list.append(sk)
                vk = f(f"vd{k}")
                nc.scalar.sqrt(vk, sk)
                vd_list.append(vk)

            # t_k = s_k + 2 (c_a - c_b) + nre2   (clamped at 0), tr_k = sqrt
            for k, (ca, cb) in enumerate(((cl, cr), (cnl, cr), (cl, cnr))):
                tk = f(f"t{k}")
                nc.vector.tensor_tensor(out=tk, in0=ca, in1=cb, op=ALU.subtract)
                nc.vector.scalar_tensor_tensor(
                    tk, tk, 2.0, s_list[k], op0=ALU.mult, op1=ALU.add
                )
                nc.vector.tensor_tensor(out=tk, in0=tk, in1=q, op=ALU.add)
                nc.vector.tensor_scalar_max(tk, tk, 0.0)
                trk = f(f"tr{k}")
                nc.scalar.sqrt(trk, tk)
                tr_list.append(trk)

            # mt = 1 - mh - mr - ms
            mt = f("mt")
            nc.vector.tensor_tensor(out=mt, in0=mh, in1=mr, op=ALU.add)
            nc.vector.tensor_tensor(out=mt, in0=mt, in1=ms, op=ALU.add)
            nc.vector.tensor_scalar(
                mt, mt, -1.0, 1.0, op0=ALU.mult, op1=ALU.add
            )

            scores = []
            for k, (ba, bb) in enumerate(((bl, br), (bnl, br), (bl, bnr))):
                vd, tr = vd_list[k], tr_list[k]
                bd = f("bd")
                nc.vector.tensor_tensor(out=bd, in0=ba, in1=bb, op=ALU.subtract)
                hyp = f("hyp")
                nc.vector.tensor_tensor(out=hyp, in0=vd, in1=bd, op=ALU.subtract)
                nc.vector.tensor_scalar_max(hyp, hyp, 0.0)
                hyr = f("hyr")
                nc.vector.tensor_tensor(out=hyr, in0=vd, in1=bd, op=ALU.add)
                nc.vector.tensor_scalar_max(hyr, hyr, 0.0)
                syn = f("syn")
                nc.vector.scalar_tensor_tensor(
                    syn, bd, -1.0, bd, op0=ALU.mult, op1=ALU.max
                )
                nc.vector.tensor_tensor(out=syn, in0=syn, in1=vd, op=ALU.add)
                sc = f(f"sc{k}")
                nc.vector.tensor_tensor(out=sc, in0=mh, in1=hyp, op=ALU.mult)
                nc.vector.tensor_tensor(out=hyp, in0=mr, in1=hyr, op=ALU.mult)
                nc.vector.tensor_tensor(out=sc, in0=sc, in1=hyp, op=ALU.add)
                nc.vector.tensor_tensor(out=hyp, in0=ms, in1=syn, op=ALU.mult)
                nc.vector.tensor_tensor(out=sc, in0=sc, in1=hyp, op=ALU.add)
                nc.vector.tensor_tensor(out=hyp, in0=mt, in1=tr, op=ALU.mult)
                nc.vector.tensor_tensor(out=sc, in0=sc, in1=hyp, op=ALU.add)
                scores.append(sc)

            q2, q3 = f("q2"), f("q3")
            nc.vector.tensor_tensor(out=q2, in0=scores[0], in1=scores[1], op=ALU.subtract)
            nc.vector.tensor_scalar(
                q2, q2, MARGIN, 0.0, op0=ALU.add, op1=ALU.max
            )
            nc.vector.tensor_tensor(out=q3, in0=scores[0], in1=scores[2], op=ALU.subtract)
            nc.vector.tensor_scalar(
                q3, q3, MARGIN, 0.0, op0=ALU.add, op1=ALU.max
            )
            nc.vector.tensor_tensor(out=q2, in0=q2, in1=q3, op=ALU.add)
            part = fpool.tile([P, 1], F32, name="part")
            nc.vector.tensor_reduce(out=part[:], in_=q2, axis=AX.X, op=ALU.add)
            nc.sync.dma_start(out=out_t[:], in_=part[:])

    nc.finalize()
    return nc


_NC_CACHE = {}


def _get_nc(nb=NB):
    if nb not in _NC_CACHE:
        _NC_CACHE[nb] = build_bass(nb)
    return _NC_CACHE[nb]


def _fused_table(inputs):
    vec = np.asarray(inputs["predVec"], dtype=np.float32)
    biasv = np.asarray(inputs["predBias"], dtype=np.float32).reshape(NUM_ENTITY)
    relemb = np.asarray(inputs["relEmb"], dtype=np.float32)

    fused = np.zeros((NUM_ENTITY, EW), dtype=np.float16)
    fused[:, 0:D] = vec.astype(np.float16)
    fused[:, OFF_BIAS] = biasv.astype(np.float16)
    fused[:, OFF_NORM] = (vec * vec).sum(axis=1).astype(np.float16)
    fused[:, OFF_R : OFF_R + NUM_RELATION] = (vec @ relemb.T).astype(np.float16)
    return fused


def _rel_meta(inputs):
    relemb = np.asarray(inputs["relEmb"], dtype=np.float32)
    rids = np.arange(NUM_RELATION)
    relf = np.zeros((NUM_RELATION, RW), dtype=np.float16)
    relf[:, 0] = np.isin(rids, HYPONYM).astype(np.float16)
    relf[:, 1] = np.isin(rids, HYPERNYM).astype(np.float16)
    relf[:, 2] = np.isin(rids, SYNONYM).astype(np.float16)
    relf[:, 3] = (relemb * relemb).sum(axis=1).astype(np.float16)
    relf[:, OFF_H : OFF_H + NUM_RELATION] = np.eye(NUM_RELATION, dtype=np.float16)
    return relf


def _prep_inputs(inputs, nb=NB, n_cores=N_CORES):
    fused = _fused_table(inputs)
    relf = _rel_meta(inputs)
    nbk = nb // P

    def shard(name):
        arr = np.asarray(inputs[name], dtype=np.int32)
        return [
            np.ascontiguousarray(arr[c * nb:(c + 1) * nb].reshape(P, nbk))
            for c in range(n_cores)
        ]

    li = shard("leftEnIndices")
    ri = shard("rightEnIndices")
    nli = shard("negLeftEnIndices")
    nri = shard("negRightEnIndices")

    rel = np.asarray(inputs["relIndices"], dtype=np.int64)
    relm = [
        np.ascontiguousarray(
            relf[rel[c * nb:(c + 1) * nb].reshape(P, nbk)].reshape(P, nbk * RW)
        )
        for c in range(n_cores)
    ]

    return [
        {
            "vec": fused, "relmeta": relm[c],
            "li": li[c], "ri": ri[c], "nli": nli[c], "nri": nri[c],
        }
        for c in range(n_cores)
    ]


def run(inputs, trace=False):
    nc = _get_nc(NB)
    in_maps = _prep_inputs(inputs)
    res = run_bass_kernel_spmd(nc, in_maps, core_ids=list(range(N_CORES)), trace=trace)
    total = sum(float(r["psum_out"].astype(np.float64).sum()) for r in res.results)
    out = np.float32(total / B)
    return np.asarray(out, dtype=np.float32), res


def kernel(**inputs) -> np.ndarray:
    out, _ = run(inputs, trace=False)
    return out


# revision 9
# speedup vs baseline: 1.0003x; 1.0003x over previous
"""Trainium2 Bass kernel for nn_NewModel_66176856097442 (TransE-style loss).

Strategy (data-parallel over the batch of triples):
  - B = 262144 triples sharded as 32768/core across 8 NeuronCores,
    laid out [128 partitions x 256 columns] per core.
  - Entity table fused on host into fp16 rows of 152 elems (304B):
      [ vec(128) | bias | ||vec||^2 | R(18) | pad(4) ]
    where R[e,k] = vec[e] . relEmb[k].  With per-entity norms and R, no
    per-triple elementwise vector arithmetic is needed:
      ||a - b||^2           = n_a + n_b - 2 a.b
      ||a + r_rel - b||^2   = ||a-b||^2 + 2(R[a,rel] - R[b,rel]) + ||r_rel||^2
    so the only per-triple vector work is the three cross dots
    (lv.rv, nlv.rv, lv.nrv), each a fused mul+reduce (tensor_tensor_reduce)
    per 128-triple column.
  - Rel-side per-triple meta (masks, ||relEmb||^2, onehot(18)) is expanded on
    host (18-entry lookup) and uploaded as one sequential tile per core.
  - Gathers: one gpsimd indirect DMA per (array, column) writing into a
    chunk-sized buffer slice; hardware only supports one index per partition
    per indirect DMA, so 4*256 calls/core.  Compute is batched per chunk of
    CH columns on large strided views.
  - Final margin/mask algebra runs once on [128, 256] tiles; per-core
    partial sum returned as [128,1]; host sums / B.
"""

import sys

sys.path.insert(0, "/opt/trn_rl_repo")

import numpy as np

import concourse.bass as bass
from concourse import bacc
import concourse.tile as tile
from concourse import mybir
from concourse.bass import IndirectOffsetOnAxis
from concourse.bass_utils import run_bass_kernel_spmd

F32 = mybir.dt.float32
F16 = mybir.dt.float16
I32 = mybir.dt.int32
ALU = mybir.AluOpType
AX = mybir.AxisListType

NUM_ENTITY = 100000
NUM_RELATION = 18
D = 128
B = 262144
N_CORES = 8
NB = B // N_CORES          # triples per core (32768)
P = 128                    # partitions
NBK = NB // P              # triples per partition per core (256)
CH = 32                    # columns per compute chunk
MARGIN = 1.0

# fused entity row: [vec(128) | bias | norm | R(18) | pad] -> 152 fp16 = 304B
EW = 152
OFF_BIAS = 128
OFF_NORM = 129
OFF_R = 130
# host-expanded rel meta per triple: [mh | mr | ms | nre2 | onehot(18) | pad]
RW = 24
OFF_H = 4

HYPONYM = (4, 6)
HYPERNYM = (3, 5)
SYNONYM = (0, 1, 13, 17)


def build_bass(nb=NB):
    """Per-core Bass kernel; nb = triples handled by this core."""
    nbk = nb // P
    assert nbk % CH == 0
    nch = nbk // CH

    nc = bacc.Bacc("TRN2", target_bir_lowering=False, debug=True)

    vec_t = nc.declare_dram_parameter("vec", [NUM_ENTITY, EW], F16, isOutput=False)
    relm_t = nc.declare_dram_parameter("relmeta", [P, nbk * RW], F16, isOutput=False)
    li_t = nc.declare_dram_parameter("li", [P, nbk], I32, isOutput=False)
    ri_t = nc.declare_dram_parameter("ri", [P, nbk], I32, isOutput=False)
    nli_t = nc.declare_dram_parameter("nli", [P, nbk], I32, isOutput=False)
    nri_t = nc.declare_dram_parameter("nri", [P, nbk], I32, isOutput=False)
    out_t = nc.declare_dram_parameter("psum_out", [P, 1], F32, isOutput=True)

    with tile.TileContext(nc) as tc:
        with (
            tc.tile_pool(name="persist", bufs=1) as persist,
            tc.tile_pool(name="gather", bufs=2) as gpool,
            tc.tile_pool(name="scratch", bufs=2) as spool,
            tc.tile_pool(name="final", bufs=1) as fpool,
        ):
            # ---- load index arrays + rel meta to SBUF once ----
            li = persist.tile([P, nbk], I32, name="li")
            ri = persist.tile([P, nbk], I32, name="ri")
            nli = persist.tile([P, nbk], I32, name="nli")
            nri = persist.tile([P, nbk], I32, name="nri")
            relm = persist.tile([P, nbk * RW], F16, name="relm")
            nc.sync.dma_start(out=li[:], in_=li_t[:])
            nc.sync.dma_start(out=ri[:], in_=ri_t[:])
            nc.sync.dma_start(out=nli[:], in_=nli_t[:])
            nc.sync.dma_start(out=nri[:], in_=nri_t[:])
            nc.sync.dma_start(out=relm[:], in_=relm_t[:])
            relm3 = relm[:].rearrange("p (c w) -> p c w", c=nbk, w=RW)

            # per-triple accumulators (f32) and extracted scalars (f16)
            dots = [persist.tile([P, nbk], F32, name=f"d{k}") for k in range(3)]
            conts = [persist.tile([P, nbk], F32, name=f"c{k}") for k in range(4)]
            exn = [persist.tile([P, nbk], F16, name=f"n{k}") for k in range(4)]
            exb = [persist.tile([P, nbk], F16, name=f"b{k}") for k in range(4)]

            idxs = (li, ri, nli, nri)
            for c in range(nch):
                j0 = c * CH
                jsl = slice(j0, j0 + CH)
                gts = []
                for a, (ixt, anm) in enumerate(zip(idxs, ("lv", "rv", "nlv", "nrv"))):
                    gt = gpool.tile([P, CH * EW], F16, name=anm, tag=anm)
                    for j in range(CH):
                        nc.gpsimd.indirect_dma_start(
                            out=gt[:, j * EW : (j + 1) * EW],
                            out_offset=None,
                            in_=vec_t[:],
                            in_offset=IndirectOffsetOnAxis(
                                ap=ixt[:, j0 + j : j0 + j + 1], axis=0
                            ),
                        )
                    gts.append(gt[:].rearrange("p (c w) -> p c w", c=CH, w=EW))

                # ---- extract per-triple scalars into packed persist tiles ----
                for a in range(4):
                    nc.vector.tensor_copy(
                        exn[a][:, jsl],
                        gts[a][:, :, OFF_NORM : OFF_NORM + 1].squeeze(),
                    )
                    nc.vector.tensor_copy(
                        exb[a][:, jsl],
                        gts[a][:, :, OFF_BIAS : OFF_BIAS + 1].squeeze(),
                    )

                # ---- cross dots: batched mul + 3D-strided reduce per pair ----
                for k, (a, b) in enumerate(((0, 1), (2, 1), (0, 3))):
                    prod = spool.tile([P, CH * D], F16, name=f"dp{k}", tag=f"dp{k}")
                    dp3 = prod[:].rearrange("p (c w) -> p c w", c=CH, w=D)
                    nc.vector.tensor_tensor(
                        out=dp3,
                        in0=gts[a][:, :, 0:D],
                        in1=gts[b][:, :, 0:D],
                        op=ALU.mult,
                    )
                    nc.vector.tensor_reduce(
                        out=dots[k][:, jsl], in_=dp3, axis=AX.X, op=ALU.add
                    )

                # ---- onehot . R contraction per entity array ----
                hview = relm3[:, jsl, OFF_H : OFF_H + NUM_RELATION]
                for a in range(4):
                    prod = spool.tile(
                        [P, CH * NUM_RELATION], F16, name=f"pr{a}", tag=f"pr{a}"
                    )
                    p3 = prod[:].rearrange("p (c w) -> p c w", c=CH, w=NUM_RELATION)
                    nc.vector.tensor_tensor(
                        out=p3,
                        in0=hview,
                        in1=gts[a][:, :, OFF_R : OFF_R + NUM_RELATION],
                        op=ALU.mult,
                    )
                    nc.vector.tensor_reduce(
                        out=conts[a][:, jsl], in_=p3, axis=AX.X, op=ALU.add
                    )

            # ================= final phase on [P, nbk] tiles =================
            f = lambda nm: fpool.tile([P, nbk], F32, name=nm)

            nl_, nr_, nnl, nnr = (t[:] for t in exn)
            bl, br, bnl, bnr = (t[:] for t in exb)
            mh = relm3[:, :, 0:1].squeeze()
            mr = relm3[:, :, 1:2].squeeze()
            ms = relm3[:, :, 2:3].squeeze()
            q = relm3[:, :, 3:4].squeeze()
            cl, cr, cnl, cnr = (t[:] for t in conts)

            # s_k = n_a + n_b - 2 d_k   (clamped at 0)
            s_list, vd_list, tr_list = [], [], []
            for k, (na, nb_) in enumerate(((nl_, nr_), (nnl, nr_), (nl_, nnr))):
                sk = f(f"s{k}")
                nc.vector.tensor_tensor(out=sk, in0=na, in1=nb_, op=ALU.add)
                nc.vector.scalar_tensor_tensor(
                    sk, dots[k][:], -2.0, sk, op0=ALU.mult, op1=ALU.add
                )
                nc.vector.tensor_scalar_max(sk, sk, 0.0)
                s_list.append(sk)
                vk = f(f"vd{k}")
                nc.scalar.sqrt(vk, sk)
                vd_list.append(vk)

            # t_k = s_k + 2 (c_a - c_b) + nre2   (clamped at 0), tr_k = sqrt
            for k, (ca, cb) in enumerate(((cl, cr), (cnl, cr), (cl, cnr))):
                tk = f(f"t{k}")
                nc.vector.tensor_tensor(out=tk, in0=ca, in1=cb, op=ALU.subtract)
                nc.vector.scalar_tensor_tensor(
                    tk, tk, 2.0, s_list[k], op0=ALU.mult, op1=ALU.add
                )
                nc.vector.tensor_tensor(out=tk, in0=tk, in1=q, op=ALU.add)
                nc.vector.tensor_scalar_max(tk, tk, 0.0)
                trk = f(f"tr{k}")
                nc.scalar.sqrt(trk, tk)
                tr_list.append(trk)

            # mt = 1 - mh - mr - ms
            mt = f("mt")
            nc.vector.tensor_tensor(out=mt, in0=mh, in1=mr, op=ALU.add)
            nc.vector.tensor_tensor(out=mt, in0=mt, in1=ms, op=ALU.add)
            nc.vector.tensor_scalar(
                mt, mt, -1.0, 1.0, op0=ALU.mult, op1=ALU.add
            )

            scores = []
            for k, (ba, bb) in enumerate(((bl, br), (bnl, br), (bl, bnr))):
                vd, tr = vd_list[k], tr_list[k]
                bd = f("bd")
                nc.vector.tensor_tensor(out=bd, in0=ba, in1=bb, op=ALU.subtract)
                hyp = f("hyp")
                nc.vector.tensor_tensor(out=hyp, in0=vd, in1=bd, op=ALU.subtract)
                nc.vector.tensor_scalar_max(hyp, hyp, 0.0)
                hyr = f("hyr")
                nc.vector.tensor_tensor(out=hyr, in0=vd, in1=bd, op=ALU.add)
                nc.vector.tensor_scalar_max(hyr, hyr, 0.0)
                syn = f("syn")
                nc.vector.scalar_tensor_tensor(
                    syn, bd, -1.0, bd, op0=ALU.mult, op1=ALU.max
                )
                nc.vector.tensor_tensor(out=syn, in0=syn, in1=vd, op=ALU.add)
                sc = f(f"sc{k}")
                nc.vector.tensor_tensor(out=sc, in0=mh, in1=hyp, op=ALU.mult)
                nc.vector.tensor_tensor(out=hyp, in0=mr, in1=hyr, op=ALU.mult)
                nc.vector.tensor_tensor(out=sc, in0=sc, in1=hyp, op=ALU.add)
                nc.vector.tensor_tensor(out=hyp, in0=ms, in1=syn, op=ALU.mult)
                nc.vector.tensor_tensor(out=sc, in0=sc, in1=hyp, op=ALU.add)
                nc.vector.tensor_tensor(out=hyp, in0=mt, in1=tr, op=ALU.mult)
                nc.vector.tensor_tensor(out=sc, in0=sc, in1=hyp, op=ALU.add)
                scores.append(sc)

            q2, q3 = f("q2"), f("q3")
            nc.vector.tensor_tensor(out=q2, in0=scores[0], in1=scores[1], op=ALU.subtract)
            nc.vector.tensor_scalar(
                q2, q2, MARGIN, 0.0, op0=ALU.add, op1=ALU.max
            )
            nc.vector.tensor_tensor(out=q3, in0=scores[0], in1=scores[2], op=ALU.subtract)
            nc.vector.tensor_scalar(
                q3, q3, MARGIN, 0.0, op0=ALU.add, op1=ALU.max
            )
            nc.vector.tensor_tensor(out=q2, in0=q2, in1=q3, op=ALU.add)
            part = fpool.tile([P, 1], F32, name="part")
            nc.vector.tensor_reduce(out=part[:], in_=q2, axis=AX.X, op=ALU.add)
            nc.sync.dma_start(out=out_t[:], in_=part[:])

    nc.finalize()
    return nc


_NC_CACHE = {}


def _get_nc(nb=NB):
    if nb not in _NC_CACHE:
        _NC_CACHE[nb] = build_bass(nb)
    return _NC_CACHE[nb]


def _fused_table(inputs):
    vec = np.asarray(inputs["predVec"], dtype=np.float32)
    biasv = np.asarray(inputs["predBias"], dtype=np.float32).reshape(NUM_ENTITY)
    relemb = np.asarray(inputs["relEmb"], dtype=np.float32)

    fused = np.zeros((NUM_ENTITY, EW), dtype=np.float16)
    fused[:, 0:D] = vec.astype(np.float16)
    fused[:, OFF_BIAS] = biasv.astype(np.float16)
    fused[:, OFF_NORM] = (vec * vec).sum(axis=1).astype(np.float16)
    fused[:, OFF_R : OFF_R + NUM_RELATION] = (vec @ relemb.T).astype(np.float16)
    return fused


def _rel_meta(inputs):
    relemb = np.asarray(inputs["relEmb"], dtype=np.float32)
    rids = np.arange(NUM_RELATION)
    relf = np.zeros((NUM_RELATION, RW), dtype=np.float16)
    relf[:, 0] = np.isin(rids, HYPONYM).astype(np.float16)
    relf[:, 1] = np.isin(rids, HYPERNYM).astype(np.float16)
    relf[:, 2] = np.isin(rids, SYNONYM).astype(np.float16)
    relf[:, 3] = (relemb * relemb).sum(axis=1).astype(np.float16)
    relf[:, OFF_H : OFF_H + NUM_RELATION] = np.eye(NUM_RELATION, dtype=np.float16)
    return relf


def _prep_inputs(inputs, nb=NB, n_cores=N_CORES):
    fused = _fused_table(inputs)
    relf = _rel_meta(inputs)
    nbk = nb // P

    def shard(name):
        arr = np.asarray(inputs[name], dtype=np.int32)
        return [
            np.ascontiguousarray(arr[c * nb:(c + 1) * nb].reshape(P, nbk))
            for c in range(n_cores)
        ]

    li = shard("leftEnIndices")
    ri = shard("rightEnIndices")
    nli = shard("negLeftEnIndices")
    nri = shard("negRightEnIndices")

    rel = np.asarray(inputs["relIndices"], dtype=np.int64)
    relm = [
        np.ascontiguousarray(
            relf[rel[c * nb:(c + 1) * nb].reshape(P, nbk)].reshape(P, nbk * RW)
        )
        for c in range(n_cores)
    ]

    return [
        {
            "vec": fused, "relmeta": relm[c],
            "li": li[c], "ri": ri[c], "nli": nli[c], "nri": nri[c],
        }
        for c in range(n_cores)
    ]


def run(inputs, trace=False):
    nc = _get_nc(NB)
    in_maps = _prep_inputs(inputs)
    res = run_bass_kernel_spmd(nc, in_maps, core_ids=list(range(N_CORES)), trace=trace)
    total = sum(float(r["psum_out"].astype(np.float64).sum()) for r in res.results)
    out = np.float32(total / B)
    return np.asarray(out, dtype=np.float32), res


def kernel(**inputs) -> np.ndarray:
    out, _ = run(inputs, trace=False)
    return out


# revision 10
# speedup vs baseline: 1.0006x; 1.0003x over previous
"""Trainium2 Bass kernel for nn_NewModel_66176856097442 (TransE-style loss).

Strategy (data-parallel over the batch of triples):
  - B = 262144 triples sharded as 32768/core across 8 NeuronCores,
    laid out [128 partitions x 256 columns] per core.
  - Entity table fused on host into fp16 rows of 152 elems (304B):
      [ vec(128) | bias | ||vec||^2 | R(18) | pad(4) ]
    where R[e,k] = vec[e] . relEmb[k].  With per-entity norms and R, no
    per-triple elementwise vector arithmetic is needed:
      ||a - b||^2           = n_a + n_b - 2 a.b
      ||a + r_rel - b||^2   = ||a-b||^2 + 2(R[a,rel] - R[b,rel]) + ||r_rel||^2
    so the only per-triple vector work is the three cross dots
    (lv.rv, nlv.rv, lv.nrv), each a fused mul+reduce (tensor_tensor_reduce)
    per 128-triple column.
  - Rel-side per-triple meta (masks, ||relEmb||^2, onehot(18)) is expanded on
    host (18-entry lookup) and uploaded as one sequential tile per core.
  - Gathers: one gpsimd indirect DMA per (array, column) writing into a
    chunk-sized buffer slice; hardware only supports one index per partition
    per indirect DMA, so 4*256 calls/core.  Compute is batched per chunk of
    CH columns on large strided views.
  - Final margin/mask algebra runs once on [128, 256] tiles; per-core
    partial sum returned as [128,1]; host sums / B.
"""

import sys

sys.path.insert(0, "/opt/trn_rl_repo")

import numpy as np

import concourse.bass as bass
from concourse import bacc
import concourse.tile as tile
from concourse import mybir
from concourse.bass import IndirectOffsetOnAxis
from concourse.bass_utils import run_bass_kernel_spmd

F32 = mybir.dt.float32
F16 = mybir.dt.float16
I32 = mybir.dt.int32
ALU = mybir.AluOpType
AX = mybir.AxisListType

NUM_ENTITY = 100000
NUM_RELATION = 18
D = 128
B = 262144
N_CORES = 8
NB = B // N_CORES          # triples per core (32768)
P = 128                    # partitions
NBK = NB // P              # triples per partition per core (256)
CH = 32                    # columns per compute chunk
MARGIN = 1.0

# fused entity row: [vec(128) | bias | norm | R(18) | pad] -> 152 fp16 = 304B
EW = 152
OFF_BIAS = 128
OFF_NORM = 129
OFF_R = 130
# host-expanded rel meta per triple: [mh | mr | ms | nre2 | onehot(18) | pad]
RW = 24
OFF_H = 4

HYPONYM = (4, 6)
HYPERNYM = (3, 5)
SYNONYM = (0, 1, 13, 17)


def build_bass(nb=NB):
    """Per-core Bass kernel; nb = triples handled by this core."""
    nbk = nb // P
    assert nbk % CH == 0
    nch = nbk // CH

    nc = bacc.Bacc("TRN2", target_bir_lowering=False, debug=True)

    vec_t = nc.declare_dram_parameter("vec", [NUM_ENTITY, EW], F16, isOutput=False)
    relm_t = nc.declare_dram_parameter("relmeta", [P, nbk * RW], F16, isOutput=False)
    li_t = nc.declare_dram_parameter("li", [P, nbk], I32, isOutput=False)
    ri_t = nc.declare_dram_parameter("ri", [P, nbk], I32, isOutput=False)
    nli_t = nc.declare_dram_parameter("nli", [P, nbk], I32, isOutput=False)
    nri_t = nc.declare_dram_parameter("nri", [P, nbk], I32, isOutput=False)
    out_t = nc.declare_dram_parameter("psum_out", [P, 1], F32, isOutput=True)

    with tile.TileContext(nc) as tc:
        with (
            tc.tile_pool(name="persist", bufs=1) as persist,
            tc.tile_pool(name="gather", bufs=3) as gpool,
            tc.tile_pool(name="scratch", bufs=2) as spool,
            tc.tile_pool(name="final", bufs=1) as fpool,
        ):
            # ---- load index arrays + rel meta to SBUF once ----
            li = persist.tile([P, nbk], I32, name="li")
            ri = persist.tile([P, nbk], I32, name="ri")
            nli = persist.tile([P, nbk], I32, name="nli")
            nri = persist.tile([P, nbk], I32, name="nri")
            relm = persist.tile([P, nbk * RW], F16, name="relm")
            nc.sync.dma_start(out=li[:], in_=li_t[:])
            nc.sync.dma_start(out=ri[:], in_=ri_t[:])
            nc.sync.dma_start(out=nli[:], in_=nli_t[:])
            nc.sync.dma_start(out=nri[:], in_=nri_t[:])
            nc.sync.dma_start(out=relm[:], in_=relm_t[:])
            relm3 = relm[:].rearrange("p (c w) -> p c w", c=nbk, w=RW)

            # per-triple accumulators (f32) and extracted scalars (f16)
            dots = [persist.tile([P, nbk], F32, name=f"d{k}") for k in range(3)]
            conts = [persist.tile([P, nbk], F32, name=f"c{k}") for k in range(4)]
            exn = [persist.tile([P, nbk], F16, name=f"n{k}") for k in range(4)]
            exb = [persist.tile([P, nbk], F16, name=f"b{k}") for k in range(4)]

            idxs = (li, ri, nli, nri)
            for c in range(nch):
                j0 = c * CH
                jsl = slice(j0, j0 + CH)
                gts = []
                for a, (ixt, anm) in enumerate(zip(idxs, ("lv", "rv", "nlv", "nrv"))):
                    gt = gpool.tile([P, CH * EW], F16, name=anm, tag=anm)
                    for j in range(CH):
                        nc.gpsimd.indirect_dma_start(
                            out=gt[:, j * EW : (j + 1) * EW],
                            out_offset=None,
                            in_=vec_t[:],
                            in_offset=IndirectOffsetOnAxis(
                                ap=ixt[:, j0 + j : j0 + j + 1], axis=0
                            ),
                        )
                    gts.append(gt[:].rearrange("p (c w) -> p c w", c=CH, w=EW))

                # ---- extract per-triple scalars into packed persist tiles ----
                for a in range(4):
                    nc.vector.tensor_copy(
                        exn[a][:, jsl],
                        gts[a][:, :, OFF_NORM : OFF_NORM + 1].squeeze(),
                    )
                    nc.vector.tensor_copy(
                        exb[a][:, jsl],
                        gts[a][:, :, OFF_BIAS : OFF_BIAS + 1].squeeze(),
                    )

                # ---- cross dots: batched mul + 3D-strided reduce per pair ----
                for k, (a, b) in enumerate(((0, 1), (2, 1), (0, 3))):
                    prod = spool.tile([P, CH * D], F16, name="dp", tag="dp")
                    dp3 = prod[:].rearrange("p (c w) -> p c w", c=CH, w=D)
                    nc.vector.tensor_tensor(
                        out=dp3,
                        in0=gts[a][:, :, 0:D],
                        in1=gts[b][:, :, 0:D],
                        op=ALU.mult,
                    )
                    nc.vector.tensor_reduce(
                        out=dots[k][:, jsl], in_=dp3, axis=AX.X, op=ALU.add
                    )

                # ---- onehot . R contraction per entity array ----
                hview = relm3[:, jsl, OFF_H : OFF_H + NUM_RELATION]
                for a in range(4):
                    prod = spool.tile(
                        [P, CH * NUM_RELATION], F16, name="pr", tag="pr"
                    )
                    p3 = prod[:].rearrange("p (c w) -> p c w", c=CH, w=NUM_RELATION)
                    nc.vector.tensor_tensor(
                        out=p3,
                        in0=hview,
                        in1=gts[a][:, :, OFF_R : OFF_R + NUM_RELATION],
                        op=ALU.mult,
                    )
                    nc.vector.tensor_reduce(
                        out=conts[a][:, jsl], in_=p3, axis=AX.X, op=ALU.add
                    )

            # ================= final phase on [P, nbk] tiles =================
            f = lambda nm: fpool.tile([P, nbk], F32, name=nm)

            nl_, nr_, nnl, nnr = (t[:] for t in exn)
            bl, br, bnl, bnr = (t[:] for t in exb)
            mh = relm3[:, :, 0:1].squeeze()
            mr = relm3[:, :, 1:2].squeeze()
            ms = relm3[:, :, 2:3].squeeze()
            q = relm3[:, :, 3:4].squeeze()
            cl, cr, cnl, cnr = (t[:] for t in conts)

            # s_k = n_a + n_b - 2 d_k   (clamped at 0)
            s_list, vd_list, tr_list = [], [], []
            for k, (na, nb_) in enumerate(((nl_, nr_), (nnl, nr_), (nl_, nnr))):
                sk = f(f"s{k}")
                nc.vector.tensor_tensor(out=sk, in0=na, in1=nb_, op=ALU.add)
                nc.vector.scalar_tensor_tensor(
                    sk, dots[k][:], -2.0, sk, op0=ALU.mult, op1=ALU.add
                )
                nc.vector.tensor_scalar_max(sk, sk, 0.0)
                s_list.append(sk)
                vk = f(f"vd{k}")
                nc.scalar.sqrt(vk, sk)
                vd_list.append(vk)

            # t_k = s_k + 2 (c_a - c_b) + nre2   (clamped at 0), tr_k = sqrt
            for k, (ca, cb) in enumerate(((cl, cr), (cnl, cr), (cl, cnr))):
                tk = f(f"t{k}")
                nc.vector.tensor_tensor(out=tk, in0=ca, in1=cb, op=ALU.subtract)
                nc.vector.scalar_tensor_tensor(
                    tk, tk, 2.0, s_list[k], op0=ALU.mult, op1=ALU.add
                )
                nc.vector.tensor_tensor(out=tk, in0=tk, in1=q, op=ALU.add)
                nc.vector.tensor_scalar_max(tk, tk, 0.0)
                trk = f(f"tr{k}")
                nc.scalar.sqrt(trk, tk)
                tr_list.append(trk)

            # mt = 1 - mh - mr - ms
            mt = f("mt")
            nc.vector.tensor_tensor(out=mt, in0=mh, in1=mr, op=ALU.add)
            nc.vector.tensor_tensor(out=mt, in0=mt, in1=ms, op=ALU.add)
            nc.vector.tensor_scalar(
                mt, mt, -1.0, 1.0, op0=ALU.mult, op1=ALU.add
            )

            scores = []
            bd = f("bd")
            hyp = f("hyp")
            hyr = f("hyr")
            syn = f("syn")
            for k, (ba, bb) in enumerate(((bl, br), (bnl, br), (bl, bnr))):
                vd, tr = vd_list[k], tr_list[k]
                nc.vector.tensor_tensor(out=bd, in0=ba, in1=bb, op=ALU.subtract)
                nc.vector.tensor_tensor(out=hyp, in0=vd, in1=bd, op=ALU.subtract)
                nc.vector.tensor_scalar_max(hyp, hyp, 0.0)
                nc.vector.tensor_tensor(out=hyr, in0=vd, in1=bd, op=ALU.add)
                nc.vector.tensor_scalar_max(hyr, hyr, 0.0)
                nc.vector.scalar_tensor_tensor(
                    syn, bd, -1.0, bd, op0=ALU.mult, op1=ALU.max
                )
                nc.vector.tensor_tensor(out=syn, in0=syn, in1=vd, op=ALU.add)
                sc = f(f"sc{k}")
                nc.vector.tensor_tensor(out=sc, in0=mh, in1=hyp, op=ALU.mult)
                nc.vector.tensor_tensor(out=hyp, in0=mr, in1=hyr, op=ALU.mult)
                nc.vector.tensor_tensor(out=sc, in0=sc, in1=hyp, op=ALU.add)
                nc.vector.tensor_tensor(out=hyp, in0=ms, in1=syn, op=ALU.mult)
                nc.vector.tensor_tensor(out=sc, in0=sc, in1=hyp, op=ALU.add)
                nc.vector.tensor_tensor(out=hyp, in0=mt, in1=tr, op=ALU.mult)
                nc.vector.tensor_tensor(out=sc, in0=sc, in1=hyp, op=ALU.add)
                scores.append(sc)

            q2, q3 = f("q2"), f("q3")
            nc.vector.tensor_tensor(out=q2, in0=scores[0], in1=scores[1], op=ALU.subtract)
            nc.vector.tensor_scalar(
                q2, q2, MARGIN, 0.0, op0=ALU.add, op1=ALU.max
            )
            nc.vector.tensor_tensor(out=q3, in0=scores[0], in1=scores[2], op=ALU.subtract)
            nc.vector.tensor_scalar(
                q3, q3, MARGIN, 0.0, op0=ALU.add, op1=ALU.max
            )
            nc.vector.tensor_tensor(out=q2, in0=q2, in1=q3, op=ALU.add)
            part = fpool.tile([P, 1], F32, name="part")
            nc.vector.tensor_reduce(out=part[:], in_=q2, axis=AX.X, op=ALU.add)
            nc.sync.dma_start(out=out_t[:], in_=part[:])

    nc.finalize()
    return nc


_NC_CACHE = {}


def _get_nc(nb=NB):
    if nb not in _NC_CACHE:
        _NC_CACHE[nb] = build_bass(nb)
    return _NC_CACHE[nb]


def _fused_table(inputs):
    vec = np.asarray(inputs["predVec"], dtype=np.float32)
    biasv = np.asarray(inputs["predBias"], dtype=np.float32).reshape(NUM_ENTITY)
    relemb = np.asarray(inputs["relEmb"], dtype=np.float32)

    fused = np.zeros((NUM_ENTITY, EW), dtype=np.float16)
    fused[:, 0:D] = vec.astype(np.float16)
    fused[:, OFF_BIAS] = biasv.astype(np.float16)
    fused[:, OFF_NORM] = (vec * vec).sum(axis=1).astype(np.float16)
    fused[:, OFF_R : OFF_R + NUM_RELATION] = (vec @ relemb.T).astype(np.float16)
    return fused


def _rel_meta(inputs):
    relemb = np.asarray(inputs["relEmb"], dtype=np.float32)
    rids = np.arange(NUM_RELATION)
    relf = np.zeros((NUM_RELATION, RW), dtype=np.float16)
    relf[:, 0] = np.isin(rids, HYPONYM).astype(np.float16)
    relf[:, 1] = np.isin(rids, HYPERNYM).astype(np.float16)
    relf[:, 2] = np.isin(rids, SYNONYM).astype(np.float16)
    relf[:, 3] = (relemb * relemb).sum(axis=1).astype(np.float16)
    relf[:, OFF_H : OFF_H + NUM_RELATION] = np.eye(NUM_RELATION, dtype=np.float16)
    return relf


def _prep_inputs(inputs, nb=NB, n_cores=N_CORES):
    fused = _fused_table(inputs)
    relf = _rel_meta(inputs)
    nbk = nb // P

    def shard(name):
        arr = np.asarray(inputs[name], dtype=np.int32)
        return [
            np.ascontiguousarray(arr[c * nb:(c + 1) * nb].reshape(P, nbk))
            for c in range(n_cores)
        ]

    li = shard("leftEnIndices")
    ri = shard("rightEnIndices")
    nli = shard("negLeftEnIndices")
    nri = shard("negRightEnIndices")

    rel = np.asarray(inputs["relIndices"], dtype=np.int64)
    relm = [
        np.ascontiguousarray(
            relf[rel[c * nb:(c + 1) * nb].reshape(P, nbk)].reshape(P, nbk * RW)
        )
        for c in range(n_cores)
    ]

    return [
        {
            "vec": fused, "relmeta": relm[c],
            "li": li[c], "ri": ri[c], "nli": nli[c], "nri": nri[c],
        }
        for c in range(n_cores)
    ]


def run(inputs, trace=False):
    nc = _get_nc(NB)
    in_maps = _prep_inputs(inputs)
    res = run_bass_kernel_spmd(nc, in_maps, core_ids=list(range(N_CORES)), trace=trace)
    total = sum(float(r["psum_out"].astype(np.float64).sum()) for r in res.results)
    out = np.float32(total / B)
    return np.asarray(out, dtype=np.float32), res


def kernel(**inputs) -> np.ndarray:
    out, _ = run(inputs, trace=False)
    return out
